# revision 1
# baseline (speedup 1.0000x reference)
"""Trainium2 Bass kernel for nn_MHInrAttn (sparse_attention, b=4 s=1024 f=1024 h=16).

Strategy (8 NeuronCores):
  - The reference uses a raw .reshape(b, h, s, d_h) with NO transpose, so head h's
    Q/K/V data comes from ROWS [64h, 64h+64) of the projected [s, f] matrix.
    Sharding 2 heads per core means each core only needs 128 rows of x per batch.
  - Per core: project Q/K/V for its 128 rows (all 4 batches), run attention for its
    2 heads x 4 batches in a "transposed" orientation (scores^T [k, q]), and produce
    a partial output projection (its heads' contribution through Wo rows).
  - Host: shard inputs, run SPMD on 8 cores, sum the 8 partials, transpose, add bo.

Device-side details:
  - str_mat is host-transposed+masked (-40 fill) so it streams naturally as [k, q].
  - softmax(k-dim = partition) sums via a ones-column matmul on the PE;
    1/rowsum broadcasts across partitions via K=1 outer-product matmuls.
  - PV matmul carries an extra ones column in V to produce the second softmax's
    row sums for free; normalization is applied to the [64, 1024] PV output.
  - All matmuls run as float32r (full fp32 data, 1 cycle/row at free-dim >= 256).
"""

import numpy as np

B, S, F, H, D = 4, 1024, 1024, 16, 64
NCORES = 8
HPC = H // NCORES  # heads per core
P = 128
NEG_FILL = -40.0

_CACHE = {}


def _build_nc(mm_dt_name="float32", causal=True):
    from contextlib import ExitStack

    import concourse.bacc as bacc
    import concourse.tile as tile
    from concourse import mybir

    dt = mybir.dt
    f32 = dt.float32
    mmdt = getattr(dt, mm_dt_name)
    Exp = mybir.ActivationFunctionType.Exp

    nc = bacc.Bacc("TRN2", target_bir_lowering=False, debug=False)

    xT_d = nc.dram_tensor("xT", [B, F, P], f32, kind="ExternalInput").ap()
    str_d = nc.dram_tensor("strT", [B, HPC, S, S], f32, kind="ExternalInput").ap()
    wq_d = nc.dram_tensor("wq", [F, F], f32, kind="ExternalInput").ap()
    wk_d = nc.dram_tensor("wk", [F, F], f32, kind="ExternalInput").ap()
    wv_d = nc.dram_tensor("wv", [F, F], f32, kind="ExternalInput").ap()
    wo_d = nc.dram_tensor("wo", [P, F], f32, kind="ExternalInput").ap()
    bias_d = nc.dram_tensor("bqkv", [3, F], f32, kind="ExternalInput").ap()
    ident_d = nc.dram_tensor("ident", [P, P], f32, kind="ExternalInput").ap()
    out_d = nc.dram_tensor("outT", [B, F, S], f32, kind="ExternalOutput").ap()

    def mm(ap):
        return ap.bitcast(mmdt)

    with ExitStack() as ctx:
        tc = ctx.enter_context(tile.TileContext(nc))
        consts = ctx.enter_context(tc.tile_pool(name="consts", bufs=1))
        qtkt = ctx.enter_context(tc.tile_pool(name="qtkt", bufs=1))
        v2p = ctx.enter_context(tc.tile_pool(name="v2", bufs=1))
        outp = ctx.enter_context(tc.tile_pool(name="outp", bufs=1))
        wop = ctx.enter_context(tc.tile_pool(name="wop", bufs=1))
        dramp = ctx.enter_context(tc.tile_pool(name="dram", bufs=1, space="DRAM"))

        ident = consts.tile([P, P], f32, tag="ident", name="ident")
        nc.sync.dma_start(out=ident, in_=ident_d)
        ones_all = consts.tile([P, P], f32, tag="ones", name="ones")
        nc.vector.memset(ones_all, 1.0)
        bias_sb = consts.tile([1, 3 * F], f32, tag="bias", name="bias")
        nc.sync.dma_start(out=bias_sb, in_=bias_d.rearrange("a b -> (a b)").unsqueeze(0))
        wo_sb = wop.tile([P, F], f32, tag="wo", name="wo")
        nc.sync.dma_start(out=wo_sb, in_=wo_d)

        QT, KT, V2, OT = {}, {}, {}, {}
        for b in range(B):
            QT[b] = qtkt.tile([P, S], f32, tag=f"qt{b}", name=f"qt{b}")
            KT[b] = qtkt.tile([P, S], f32, tag=f"kt{b}", name=f"kt{b}")
            OT[b] = outp.tile([P, S], f32, tag=f"ot{b}", name=f"ot{b}")
            for hp in range(HPC):
                V2[b, hp] = v2p.tile([P, 8, P], f32, tag=f"v{b}{hp}", name=f"v{b}{hp}")

        # ---------- phase 1: projections + layout shuffles ----------
        with tc.tile_pool(name="xt", bufs=1) as xtp, \
                tc.tile_pool(name="wpool", bufs=1) as wp, \
                tc.tile_pool(name="qkvc", bufs=1) as qkvcp, \
                tc.tile_pool(name="pj", bufs=2, space="PSUM") as ppool, \
                tc.tile_pool(name="tp", bufs=2, space="PSUM") as tpool:
            xt = {}
            for b in range(B):
                xt[b] = xtp.tile([P, 8, P], f32, tag=f"xt{b}", name=f"xt{b}")
                nc.sync.dma_start(out=xt[b], in_=xT_d[b].rearrange("(kc p) r -> p kc r", p=P))

            qkvc = {}
            for t_i, w_d in enumerate([wq_d, wk_d, wv_d]):
                wt = []
                for i in range(8):
                    w_tile = wp.tile([P, F], f32, tag=f"w{i}", name=f"w{i}")
                    nc.sync.dma_start(out=w_tile, in_=w_d[i * P:(i + 1) * P, :])
                    wt.append(w_tile)
                for b in range(B):
                    cc = qkvcp.tile([P, F], f32, tag=f"c{t_i}{b}", name=f"c{t_i}{b}")
                    qkvc[t_i, b] = cc
                    for h2 in range(2):
                        ps = ppool.tile([P, 512], f32, tag="pj", name="pj")
                        for kc in range(8):
                            nc.tensor.matmul(
                                ps, mm(xt[b][:, kc, :]),
                                mm(wt[kc][:, 512 * h2:512 * (h2 + 1)]),
                                start=(kc == 0), stop=False)
                        nc.tensor.matmul(
                            ps, mm(ones_all[0:1, :]),
                            mm(bias_sb[0:1, 1024 * t_i + 512 * h2:1024 * t_i + 512 * h2 + 512]),
                            start=False, stop=True)
                        nc.scalar.copy(cc[:, 512 * h2:512 * (h2 + 1)], ps)

            # V shuffle through DRAM into [s'-chunk partitions, d] layout (+ones col)
            vs = {}
            for b in range(B):
                vs[b] = dramp.tile([P, F], f32, tag=f"vs{b}", name=f"vs{b}")
                nc.sync.dma_start(out=vs[b], in_=qkvc[2, b][:])
            for b in range(B):
                for hp in range(HPC):
                    nc.vector.memset(V2[b, hp], 0.0)
                    dcol = 64 * hp
                    ones_col = 64 if hp == 0 else 0
                    src = vs[b][64 * hp:64 * hp + 64, :].rearrange(
                        "(j r) (cb d) -> (r cb) j d", j=8, cb=16)
                    nc.sync.dma_start(out=V2[b, hp][:, :, dcol:dcol + 64], in_=src)
                    nc.vector.memset(V2[b, hp][:, :, ones_col:ones_col + 1], 1.0)

            # Q^T / K^T via 64x64 PE transposes (both heads stacked on partitions)
            for b in range(B):
                for t_i, dstmap in ((0, QT), (1, KT)):
                    for half in range(2):
                        # transpose psum outputs must be at partition 0; the
                        # DVE copy shifts head 1 back up to partitions 64-127
                        psts = []
                        for hp in range(HPC):
                            base = 64 * hp
                            pst = tpool.tile([P, 512], f32, tag=f"tp{hp}", name=f"tp{hp}")
                            psts.append(pst)
                            for cb8 in range(8):
                                cb = 8 * half + cb8
                                nc.tensor.transpose(
                                    pst[0:64, 64 * cb8:64 * cb8 + 64],
                                    qkvc[t_i, b][base:base + 64, 64 * cb:64 * cb + 64],
                                    ident[base:base + 64, base:base + 64])
                        for hp in range(HPC):
                            dst = dstmap[b][64 * hp:64 * hp + 64, :].rearrange(
                                "p (r cb) -> p cb r", cb=16)[:, 8 * half:8 * half + 8, :]
                            nc.vector.tensor_copy(
                                dst, psts[hp][0:64, :].rearrange("p (cb8 r) -> p cb8 r", cb8=8))

        # ---------- phase 2: attention ----------
        with tc.tile_pool(name="em", bufs=1) as emp, \
                tc.tile_pool(name="ep", bufs=3) as epool, \
                tc.tile_pool(name="misc", bufs=2) as miscp, \
                tc.tile_pool(name="aps", bufs=1, space="PSUM") as aps, \
                tc.tile_pool(name="qkps", bufs=2, space="PSUM") as qkps:
            for b in range(B):
                eM, r1bc = {}, {}
                for hp in range(HPC):
                    ps_r1 = [aps.tile([1, 512], f32, tag=f"r1_{h2}", name=f"r1_{h2}") for h2 in range(2)]
                    for j in range(8):
                        jl = 128 * j if causal else 0
                        w = S - jl
                        t = emp.tile([P, w], f32, tag=f"e{hp}{j}", name=f"e{hp}{j}")
                        eM[hp, j] = t
                        nc.sync.dma_start(out=t, in_=str_d[b, hp, 128 * j:128 * (j + 1), jl:])
                        nc.scalar.activation(t, t, Exp)
                        for h2 in range(2):
                            lo = max(512 * h2, jl)
                            hi = 512 * (h2 + 1)
                            if lo < hi:
                                last_j = (3 if h2 == 0 else 7) if causal else 7
                                nc.tensor.matmul(
                                    ps_r1[h2][0:1, lo - 512 * h2:hi - 512 * h2],
                                    mm(ones_all[:, 0:1]), mm(t[:, lo - jl:hi - jl]),
                                    start=(j == 0), stop=(j == last_j))
                    r1sb = miscp.tile([1, S], f32, tag=f"r1sb{hp}", name=f"r1sb{hp}")
                    rbc = miscp.tile([P, S], f32, tag=f"r1bc{hp}", name=f"r1bc{hp}")
                    r1bc[hp] = rbc
                    for h2 in range(2):
                        sl = slice(512 * h2, 512 * (h2 + 1))
                        nc.vector.reciprocal(r1sb[:, sl], ps_r1[h2])
                        psb = aps.tile([P, 512], f32, tag="bc", name="bc")
                        nc.tensor.matmul(psb, mm(ones_all[0:1, :]), mm(r1sb[0:1, sl]),
                                         start=True, stop=True)
                        nc.vector.tensor_copy(rbc[:, sl], psb)

                for hp in range(HPC):
                    base = 64 * hp
                    pv = [aps.tile([P, 512], f32, tag=f"pv{h2}", name=f"pv{h2}") for h2 in range(2)]
                    for j in range(8):
                        jl = 128 * j if causal else 0
                        Ej = epool.tile([P, S], f32, tag="E", name="E")
                        for h2 in range(2):
                            lo_h, hi_h = 512 * h2, 512 * (h2 + 1)
                            qk = qkps.tile([P, 512], f32, tag="qk", name="qk")
                            nc.tensor.matmul(
                                qk, mm(KT[b][base:base + 64, 128 * j:128 * (j + 1)]),
                                mm(QT[b][base:base + 64, lo_h:hi_h]),
                                start=True, stop=True)
                            m0_hi = min(jl, hi_h)
                            if m0_hi > lo_h:
                                nc.scalar.activation(Ej[:, lo_h:m0_hi], qk[:, 0:m0_hi - lo_h], Exp)
                            v_lo = max(jl, lo_h)
                            if v_lo < hi_h:
                                sl_E = Ej[:, v_lo:hi_h]
                                nc.vector.tensor_mul(sl_E, eM[hp, j][:, v_lo - jl:hi_h - jl],
                                                     r1bc[hp][:, v_lo:hi_h])
                                nc.vector.tensor_add(sl_E, sl_E, qk[:, v_lo - lo_h:hi_h - lo_h])
                                nc.scalar.activation(sl_E, sl_E, Exp)
                            nc.tensor.matmul(pv[h2], mm(V2[b, hp][:, j, :]), mm(Ej[:, lo_h:hi_h]),
                                             start=(j == 0), stop=(j == 7))
                    # normalize rows of PV by 1/rowsum2 (from the ones column)
                    sum_row = 64 if hp == 0 else 0
                    dlo = 64 * hp
                    r2sb = miscp.tile([P, S], f32, tag="r2sb", name="r2sb")
                    r2bc = miscp.tile([P, S], f32, tag="r2bc", name="r2bc")
                    for h2 in range(2):
                        sl = slice(512 * h2, 512 * (h2 + 1))
                        nc.vector.reciprocal(r2sb[sum_row:sum_row + 1, sl],
                                             pv[h2][sum_row:sum_row + 1, :])
                        psb = aps.tile([P, 512], f32, tag="bc", name="bc")
                        nc.tensor.matmul(psb[dlo:dlo + 64, :],
                                         mm(ones_all[sum_row:sum_row + 1, 0:64]),
                                         mm(r2sb[sum_row:sum_row + 1, sl]),
                                         start=True, stop=True)
                        nc.vector.tensor_copy(r2bc[dlo:dlo + 64, sl], psb[dlo:dlo + 64, :])
                        nc.vector.tensor_mul(OT[b][dlo:dlo + 64, sl], pv[h2][dlo:dlo + 64, :],
                                             r2bc[dlo:dlo + 64, sl])

        # ---------- phase 3: partial output projection ----------
        with tc.tile_pool(name="os", bufs=3) as osp, \
                tc.tile_pool(name="ops", bufs=2, space="PSUM") as opsum:
            for b in range(B):
                for fo in range(8):
                    ot = osp.tile([P, S], f32, tag="os", name="os")
                    for h2 in range(2):
                        ps = opsum.tile([P, 512], f32, tag="op", name="op")
                        nc.tensor.matmul(ps, mm(wo_sb[:, 128 * fo:128 * (fo + 1)]),
                                         mm(OT[b][:, 512 * h2:512 * (h2 + 1)]),
                                         start=True, stop=True)
                        nc.scalar.copy(ot[:, 512 * h2:512 * (h2 + 1)], ps)
                    nc.sync.dma_start(out=out_d[b, 128 * fo:128 * (fo + 1), :], in_=ot)

    nc.compile()
    return nc


def _prep_host(x, str_mat, attn_mask, Wq, bq, Wk, bk, Wv, bv, Wo, bo):
    x = np.asarray(x, np.float32)
    str_mat = np.asarray(str_mat, np.float32)
    attn_mask = np.asarray(attn_mask, np.float32)
    mask = attn_mask[:, 0]  # [b, s, s]
    causal = bool((mask == np.tril(np.ones((S, S), np.float32))[None]).all())
    strT = np.where(mask[:, None] == 0.0, NEG_FILL, str_mat).transpose(0, 1, 3, 2)
    xT = x.transpose(0, 2, 1)  # [b, f, s]
    Wq_s = (np.asarray(Wq, np.float32) / D)
    bq_s = (np.asarray(bq, np.float32) / D)
    bias = np.stack([bq_s, np.asarray(bk, np.float32), np.asarray(bv, np.float32)])
    ident = np.eye(P, dtype=np.float32)
    in_maps = []
    for c in range(NCORES):
        in_maps.append({
            "xT": np.ascontiguousarray(xT[:, :, P * c:P * (c + 1)]),
            "strT": np.ascontiguousarray(strT[:, HPC * c:HPC * (c + 1)]),
            "wq": Wq_s, "wk": np.asarray(Wk, np.float32), "wv": np.asarray(Wv, np.float32),
            "wo": np.ascontiguousarray(np.asarray(Wo, np.float32)[P * c:P * (c + 1)]),
            "bqkv": bias, "ident": ident,
        })
    return in_maps, causal


def kernel(**inputs):
    from concourse.bass_utils import run_bass_kernel_spmd

    in_maps, causal = _prep_host(**inputs)
    key = ("float32", causal)
    if key not in _CACHE:
        _CACHE[key] = _build_nc(mm_dt_name=key[0], causal=causal)
    nc = _CACHE[key]
    res = run_bass_kernel_spmd(nc, in_maps, core_ids=list(range(NCORES)))
    partials = [r["outT"] for r in res.results]
    out = np.sum(partials, axis=0, dtype=np.float32)  # [b, f, s]
    out = out.transpose(0, 2, 1) + np.asarray(inputs["bo"], np.float32)
    return np.ascontiguousarray(out.astype(np.float32))



# revision 20
# speedup vs baseline: 1.7791x; 1.7791x over previous
"""Trainium2 Bass kernel for nn_MHInrAttn (sparse_attention, b=4 s=1024 f=1024 h=16).

Strategy (8 NeuronCores):
  - The reference uses a raw .reshape(b, h, s, d_h) with NO transpose, so head h's
    Q/K/V data comes from ROWS [64h, 64h+64) of the projected [s, f] matrix.
    Sharding 2 heads per core means each core only needs 128 rows of x per batch.
  - Per core: project Q/K/V for its 128 rows (all 4 batches), run attention for its
    2 heads x 4 batches in a "transposed" orientation (scores^T [k, q]), and produce
    a partial output projection (its heads' contribution through Wo rows).
  - Host: shard inputs, run SPMD on 8 cores, sum the 8 partials, transpose, add bo.

Device-side details:
  - All matmul operands are bf16 (1 cycle/row on the PE, FWL weight loads,
    halved DMA); PSUM accumulation stays fp32.
  - str_mat is host-transposed+masked (-40 fill) + bf16 so it streams as [k, q].
  - softmax(k-dim = partition) sums via a ones-column matmul on the PE;
    1/rowsum (reciprocal_approx_fast) broadcasts across partitions via K=1
    outer-product matmuls and is multiplied into eM in place (PSUM operand).
  - scores bias is added into the QK PSUM tile in place, so each [128,512]
    chunk needs exactly one Exp over the full width.
  - PV matmul carries an extra ones column in V to produce the second softmax's
    row sums for free; normalization is applied to the [64, 1024] PV output.
"""

import numpy as np

B, S, F, H, D = 4, 1024, 1024, 16, 64
NCORES = 8
HPC = H // NCORES  # heads per core
P = 128
NEG_FILL = -40.0

_CACHE = {}


def _build_nc(causal=True, debug_dump=False):
    from contextlib import ExitStack

    import concourse.bacc as bacc
    import concourse.tile as tile
    from concourse import mybir

    dt = mybir.dt
    f32 = dt.float32
    bf16 = dt.bfloat16
    Exp = mybir.ActivationFunctionType.Exp

    nc = bacc.Bacc("TRN2", target_bir_lowering=False, debug=False)

    xT_d = nc.dram_tensor("xT", [B, F, P], bf16, kind="ExternalInput").ap()
    str_d = nc.dram_tensor("strT", [B, HPC, S, S], bf16, kind="ExternalInput").ap()
    wq_d = nc.dram_tensor("wq", [F, F], bf16, kind="ExternalInput").ap()
    wk_d = nc.dram_tensor("wk", [F, F], bf16, kind="ExternalInput").ap()
    wv_d = nc.dram_tensor("wv", [F, F], bf16, kind="ExternalInput").ap()
    wo_d = nc.dram_tensor("wo", [P, F], bf16, kind="ExternalInput").ap()
    bias_d = nc.dram_tensor("bqkv", [3, F], bf16, kind="ExternalInput").ap()
    ident_d = nc.dram_tensor("ident", [P, P], bf16, kind="ExternalInput").ap()
    out_d = nc.dram_tensor("outT", [B, F, S], bf16, kind="ExternalOutput").ap()
    dbg = {}
    if debug_dump:
        for nm, shp, dty in [("d_qkvc0", [P, F], bf16), ("d_qt0", [P, S], bf16),
                             ("d_kt0", [P, S], bf16), ("d_v20", [P, 8, P], bf16),
                             ("d_rbc0", [P, S], bf16), ("d_emn0", [P, S], f32),
                             ("d_ej0", [P, S], bf16), ("d_ot0", [P, S], bf16)]:
            dbg[nm] = nc.dram_tensor(nm, shp, dty, kind="ExternalOutput").ap()

    with ExitStack() as ctx:
        tc = ctx.enter_context(tile.TileContext(nc))
        consts = ctx.enter_context(tc.tile_pool(name="consts", bufs=1))
        qtkt = ctx.enter_context(tc.tile_pool(name="qtkt", bufs=1))
        v2p = ctx.enter_context(tc.tile_pool(name="v2", bufs=1))
        outp = ctx.enter_context(tc.tile_pool(name="outp", bufs=1))
        wop = ctx.enter_context(tc.tile_pool(name="wop", bufs=1))
        dramp = ctx.enter_context(tc.tile_pool(name="dram", bufs=1, space="DRAM"))

        ident = consts.tile([P, P], bf16, tag="ident", name="ident")
        nc.sync.dma_start(out=ident, in_=ident_d)
        ones_all = consts.tile([P, P], bf16, tag="ones", name="ones")
        nc.vector.memset(ones_all, 1.0)
        ones_f32 = consts.tile([P, P], f32, tag="ones32", name="ones32")
        nc.vector.memset(ones_f32, 1.0)
        bias_sb = consts.tile([1, 3 * F], bf16, tag="bias", name="bias")
        nc.sync.dma_start(out=bias_sb, in_=bias_d.rearrange("a b -> (a b)").unsqueeze(0))
        wo_sb = wop.tile([P, F], bf16, tag="wo", name="wo")
        nc.sync.dma_start(out=wo_sb, in_=wo_d)

        QT, KT, V2, OT = {}, {}, {}, {}
        for b in range(B):
            QT[b] = qtkt.tile([P, S], bf16, tag=f"qt{b}", name=f"qt{b}")
            KT[b] = qtkt.tile([P, S], bf16, tag=f"kt{b}", name=f"kt{b}")
            OT[b] = outp.tile([P, S], bf16, tag=f"ot{b}", name=f"ot{b}")
            for hp in range(HPC):
                V2[b, hp] = v2p.tile([P, 8, P], bf16, tag=f"v{b}{hp}", name=f"v{b}{hp}")

        # ---------- phase 1: projections + layout shuffles ----------
        with tc.tile_pool(name="xt", bufs=1) as xtp, \
                tc.tile_pool(name="wpool", bufs=1) as wp, \
                tc.tile_pool(name="qkvc", bufs=1) as qkvcp, \
                tc.tile_pool(name="pj", bufs=2, space="PSUM") as ppool, \
                tc.tile_pool(name="tp", bufs=2, space="PSUM") as tpool:
            xt = {}
            for b in range(B):
                xt[b] = xtp.tile([P, 8, P], bf16, tag=f"xt{b}", name=f"xt{b}")
                nc.sync.dma_start(out=xt[b], in_=xT_d[b].rearrange("(kc p) r -> p kc r", p=P))

            qkvc = {}
            for t_i, w_d in enumerate([wq_d, wk_d, wv_d]):
                wt = []
                for i in range(8):
                    w_tile = wp.tile([P, F], bf16, tag=f"w{i}", name=f"w{i}")
                    nc.sync.dma_start(out=w_tile, in_=w_d[i * P:(i + 1) * P, :])
                    wt.append(w_tile)
                for b in range(B):
                    cc = qkvcp.tile([P, F], bf16, tag=f"c{t_i}{b}", name=f"c{t_i}{b}")
                    qkvc[t_i, b] = cc
                    for h2 in range(2):
                        ps = ppool.tile([P, 512], f32, tag="pj", name="pj")
                        for kc in range(8):
                            nc.tensor.matmul(
                                ps, xt[b][:, kc, :],
                                wt[kc][:, 512 * h2:512 * (h2 + 1)],
                                start=(kc == 0), stop=False)
                        nc.tensor.matmul(
                            ps, ones_all[0:1, :],
                            bias_sb[0:1, 1024 * t_i + 512 * h2:1024 * t_i + 512 * h2 + 512],
                            start=False, stop=True)
                        nc.scalar.copy(cc[:, 512 * h2:512 * (h2 + 1)], ps)

            # V shuffle through DRAM into [s'-chunk partitions, d] layout (+ones col)
            vs = {}
            for b in range(B):
                vs[b] = dramp.tile([P, F], bf16, tag=f"vs{b}", name=f"vs{b}")
                nc.sync.dma_start(out=vs[b], in_=qkvc[2, b][:])
            # V2 column layout: col 0 = ones (PV row-sum lands at PSUM partition
            # 0, where reciprocal_approx_fast is known-good), cols 64:128 = V.
            for b in range(B):
                for hp in range(HPC):
                    nc.vector.memset(V2[b, hp], 0.0)
                    src = vs[b][64 * hp:64 * hp + 64, :].rearrange(
                        "(j r) (cb d) -> (r cb) j d", j=8, cb=16)
                    nc.sync.dma_start(out=V2[b, hp][:, :, 64:128], in_=src)
                    nc.vector.memset(V2[b, hp][:, :, 0:1], 1.0)

            if debug_dump:
                nc.sync.dma_start(out=dbg["d_qkvc0"], in_=qkvc[0, 0][:])

            # Q^T / K^T via 64x64 PE transposes (both heads stacked on partitions)
            for b in range(B):
                for t_i, dstmap in ((0, QT), (1, KT)):
                    for half in range(2):
                        # transpose psum outputs must be at partition 0; the
                        # DVE copy shifts head 1 back up to partitions 64-127
                        psts = []
                        for hp in range(HPC):
                            base = 64 * hp
                            pst = tpool.tile([P, 512], bf16, tag=f"tp{hp}", name=f"tp{hp}")
                            psts.append(pst)
                            for cb8 in range(8):
                                cb = 8 * half + cb8
                                nc.tensor.transpose(
                                    pst[0:64, 64 * cb8:64 * cb8 + 64],
                                    qkvc[t_i, b][base:base + 64, 64 * cb:64 * cb + 64],
                                    ident[base:base + 64, base:base + 64])
                        for hp in range(HPC):
                            dst = dstmap[b][64 * hp:64 * hp + 64, :].rearrange(
                                "p (r cb) -> p cb r", cb=16)[:, 8 * half:8 * half + 8, :]
                            nc.vector.tensor_copy(
                                dst, psts[hp][0:64, :].rearrange("p (cb8 r) -> p cb8 r", cb8=8))

        if debug_dump:
            nc.sync.dma_start(out=dbg["d_qt0"], in_=QT[0][:])
            nc.sync.dma_start(out=dbg["d_kt0"], in_=KT[0][:])
            nc.sync.dma_start(out=dbg["d_v20"], in_=V2[0, 0][:])

        # ---------- phase 2: attention ----------
        with tc.tile_pool(name="em", bufs=1) as emp, \
                tc.tile_pool(name="ep", bufs=3) as epool, \
                tc.tile_pool(name="misc", bufs=2) as miscp, \
                tc.tile_pool(name="aps", bufs=1, space="PSUM") as aps, \
                tc.tile_pool(name="qkps", bufs=2, space="PSUM") as qkps:
            for b in range(B):
                eM = {}
                for hp in range(HPC):
                    ps_r1 = [aps.tile([1, 512], f32, tag=f"r1_{h2}", name=f"r1_{h2}") for h2 in range(2)]
                    for j in range(8):
                        jl = 128 * j if causal else 0
                        w = S - jl
                        t = emp.tile([P, w], bf16, tag=f"e{hp}{j}", name=f"e{hp}{j}")
                        eM[hp, j] = t
                        nc.sync.dma_start(out=t, in_=str_d[b, hp, 128 * j:128 * (j + 1), jl:])
                        nc.scalar.activation(t, t, Exp)
                        for h2 in range(2):
                            lo = max(512 * h2, jl)
                            hi = 512 * (h2 + 1)
                            if lo < hi:
                                last_j = (3 if h2 == 0 else 7) if causal else 7
                                nc.tensor.matmul(
                                    ps_r1[h2][0:1, lo - 512 * h2:hi - 512 * h2],
                                    ones_all[:, 0:1], t[:, lo - jl:hi - jl],
                                    start=(j == 0), stop=(j == last_j))
                    r1sb = miscp.tile([1, S], f32, tag=f"r1sb{hp}", name=f"r1sb{hp}")
                    rbc = miscp.tile([P, S], bf16, tag=f"rbc{hp}", name=f"rbc{hp}")
                    for h2 in range(2):
                        sl = slice(512 * h2, 512 * (h2 + 1))
                        nc.vector.reciprocal_approx_fast(r1sb[:, sl], ps_r1[h2])
                        psb = aps.tile([P, 512], f32, tag=f"r1b{h2}", name=f"r1b{h2}")
                        nc.tensor.matmul(psb, ones_f32[0:1, :], r1sb[0:1, sl],
                                         start=True, stop=True)
                        nc.vector.tensor_copy(rbc[:, sl], psb)
                    if debug_dump and b == 0 and hp == 0:
                        nc.sync.dma_start(out=dbg["d_rbc0"], in_=rbc[:])
                    # normalized softmax1 bias, f32 (matches qk dtype in the add)
                    for j in range(8):
                        jl = 128 * j if causal else 0
                        t32 = emp.tile([P, S - jl if causal else S], f32,
                                       tag=f"n{hp}{j}", name=f"n{hp}{j}")
                        nc.vector.tensor_mul(t32, eM[hp, j], rbc[:, jl:])
                        eM[hp, j] = t32
                        if debug_dump and b == 0 and hp == 0 and j == 0:
                            nc.sync.dma_start(out=dbg["d_emn0"], in_=t32)

                for hp in range(HPC):
                    base = 64 * hp
                    pv = [aps.tile([P, 512], f32, tag=f"pv{h2}", name=f"pv{h2}") for h2 in range(2)]
                    for j in range(8):
                        jl = 128 * j if causal else 0
                        Ej = epool.tile([P, S], bf16, tag="E", name="E")
                        for h2 in range(2):
                            lo_h, hi_h = 512 * h2, 512 * (h2 + 1)
                            Esc = epool.tile([P, 512], f32, tag="Esc", name="Esc")
                            qk = qkps.tile([P, 512], f32, tag="qk", name="qk")
                            nc.tensor.matmul(
                                qk, KT[b][base:base + 64, 128 * j:128 * (j + 1)],
                                QT[b][base:base + 64, lo_h:hi_h],
                                start=True, stop=True)
                            m0_hi = min(jl, hi_h)
                            if m0_hi > lo_h:
                                nc.scalar.activation(Ej[:, lo_h:m0_hi],
                                                     qk[:, 0:m0_hi - lo_h], Exp)
                            v_lo = max(jl, lo_h)
                            if v_lo < hi_h:
                                w2 = hi_h - v_lo
                                nc.vector.tensor_add(
                                    Esc[:, 0:w2], eM[hp, j][:, v_lo - jl:hi_h - jl],
                                    qk[:, v_lo - lo_h:hi_h - lo_h])
                                nc.scalar.activation(Ej[:, v_lo:hi_h], Esc[:, 0:w2], Exp)
                            nc.tensor.matmul(pv[h2], V2[b, hp][:, j, :], Ej[:, lo_h:hi_h],
                                             start=(j == 0), stop=(j == 7))
                        if debug_dump and b == 0 and hp == 0 and j == 0:
                            nc.sync.dma_start(out=dbg["d_ej0"], in_=Ej[:])
                    # normalize rows of PV by 1/rowsum2 (ones column -> PSUM row 0)
                    dlo = 64 * hp
                    r2sb = miscp.tile([P, S], f32, tag="r2sb", name="r2sb")
                    r2bc = miscp.tile([P, S], f32, tag="r2bc", name="r2bc")
                    for h2 in range(2):
                        sl = slice(512 * h2, 512 * (h2 + 1))
                        nc.vector.reciprocal_approx_fast(r2sb[0:1, sl],
                                                         pv[h2][0:1, :])
                        psb = aps.tile([P, 512], f32, tag=f"r1b{h2}", name="bc")
                        nc.tensor.matmul(psb[dlo:dlo + 64, :],
                                         ones_f32[0:1, 0:64],
                                         r2sb[0:1, sl],
                                         start=True, stop=True)
                        nc.vector.tensor_copy(r2bc[dlo:dlo + 64, sl], psb[dlo:dlo + 64, :])
                        nc.vector.tensor_mul(OT[b][dlo:dlo + 64, sl], pv[h2][64:128, :],
                                             r2bc[dlo:dlo + 64, sl])

        if debug_dump:
            nc.sync.dma_start(out=dbg["d_ot0"], in_=OT[0][:])

        # ---------- phase 3: partial output projection ----------
        with tc.tile_pool(name="os", bufs=3) as osp, \
                tc.tile_pool(name="ops", bufs=4, space="PSUM") as opsum:
            for b in range(B):
                for fo in range(8):
                    ot = osp.tile([P, S], bf16, tag="os", name="os")
                    for h2 in range(2):
                        ps = opsum.tile([P, 512], f32, tag="op", name="op")
                        nc.tensor.matmul(ps, wo_sb[:, 128 * fo:128 * (fo + 1)],
                                         OT[b][:, 512 * h2:512 * (h2 + 1)],
                                         start=True, stop=True)
                        if h2 == 0:
                            nc.scalar.copy(ot[:, 512 * h2:512 * (h2 + 1)], ps)
                        else:
                            nc.vector.tensor_copy(ot[:, 512 * h2:512 * (h2 + 1)], ps)
                    nc.sync.dma_start(out=out_d[b, 128 * fo:128 * (fo + 1), :], in_=ot)

    nc.compile()
    return nc


def _prep_host(x, str_mat, attn_mask, Wq, bq, Wk, bk, Wv, bv, Wo, bo):
    import ml_dtypes
    bf = ml_dtypes.bfloat16

    x = np.asarray(x, np.float32)
    str_mat = np.asarray(str_mat, np.float32)
    attn_mask = np.asarray(attn_mask, np.float32)
    mask = attn_mask[:, 0]  # [b, s, s]
    causal = bool((mask == np.tril(np.ones((S, S), np.float32))[None]).all())
    strT = np.where(mask[:, None] == 0.0, NEG_FILL, str_mat).transpose(0, 1, 3, 2)
    strT = strT.astype(bf)
    xT = x.transpose(0, 2, 1).astype(bf)  # [b, f, s]
    Wq_s = (np.asarray(Wq, np.float32) / D).astype(bf)
    bq_s = (np.asarray(bq, np.float32) / D)
    bias = np.stack([bq_s, np.asarray(bk, np.float32),
                     np.asarray(bv, np.float32)]).astype(bf)
    Wk_b = np.asarray(Wk, np.float32).astype(bf)
    Wv_b = np.asarray(Wv, np.float32).astype(bf)
    Wo_b = np.asarray(Wo, np.float32).astype(bf)
    ident = np.eye(P, dtype=bf)
    in_maps = []
    for c in range(NCORES):
        in_maps.append({
            "xT": np.ascontiguousarray(xT[:, :, P * c:P * (c + 1)]),
            "strT": np.ascontiguousarray(strT[:, HPC * c:HPC * (c + 1)]),
            "wq": Wq_s, "wk": Wk_b, "wv": Wv_b,
            "wo": np.ascontiguousarray(Wo_b[P * c:P * (c + 1)]),
            "bqkv": bias, "ident": ident,
        })
    return in_maps, causal


def kernel(**inputs):
    from concourse.bass_utils import run_bass_kernel_spmd

    in_maps, causal = _prep_host(**inputs)
    if causal not in _CACHE:
        _CACHE[causal] = _build_nc(causal=causal)
    nc = _CACHE[causal]
    res = run_bass_kernel_spmd(nc, in_maps, core_ids=list(range(NCORES)))
    partials = [r["outT"].astype(np.float32) for r in res.results]
    out = np.sum(partials, axis=0, dtype=np.float32)  # [b, f, s]
    out = out.transpose(0, 2, 1) + np.asarray(inputs["bo"], np.float32)
    return np.ascontiguousarray(out.astype(np.float32))


# revision 30
# speedup vs baseline: 1.8434x; 1.0361x over previous
"""Trainium2 Bass kernel for nn_MHInrAttn (sparse_attention, b=4 s=1024 f=1024 h=16).

Strategy (8 NeuronCores):
  - The reference uses a raw .reshape(b, h, s, d_h) with NO transpose, so head h's
    Q/K/V data comes from ROWS [64h, 64h+64) of the projected [s, f] matrix.
    Sharding 2 heads per core means each core only needs 128 rows of x per batch.
  - Per core: project Q/K/V for its 128 rows (all 4 batches), run attention for its
    2 heads x 4 batches in a "transposed" orientation (scores^T [k, q]), and produce
    a partial output projection (its heads' contribution through Wo rows).
  - Host: shard inputs, run SPMD on 8 cores, sum the 8 partials, transpose, add bo.

Device-side details:
  - All matmul operands are bf16 (1 cycle/row on the PE, FWL weight loads,
    halved DMA); PSUM accumulation stays fp32.
  - All phases share one scope so per-batch stages pipeline: projections /
    transposes (b), attention (b), output projection + DMA-out (b) overlap
    with other batches' stages. One PSUM pool: mm(3) r1_0 r1_1 r1b pv0 pv1.
  - str_mat is host-transposed+masked (-40 fill) + bf16 so it streams as [k, q].
  - Q^T/K^T are kept in raw transpose-output order [d, (cb, r)] so the
    PSUM->SBUF copies are contiguous; the QK matmuls read them through a
    strided access pattern (q = r*16 + cb).
  - softmax(k-dim = partition) sums via a ones-column matmul on the PE;
    1/rowsum (reciprocal_approx_fast, partition 0 only) broadcasts across
    partitions via K=1 matmuls; normalized bias is materialized in f32 to
    match the QK PSUM dtype in the score add.
  - PV matmul carries a ones column in V (col 0 -> PSUM partition 0) for the
    second softmax's row sums; normalization multiplies the PV rows (64:128)
    into OT with a partition-shifting DVE op.
"""

import numpy as np

B, S, F, H, D = 4, 1024, 1024, 16, 64
NCORES = 8
HPC = H // NCORES  # heads per core
P = 128
NEG_FILL = -40.0

_CACHE = {}


def _build_nc(causal=True, debug_dump=False):
    from contextlib import ExitStack

    import concourse.bacc as bacc
    import concourse.tile as tile
    from concourse import mybir

    dt = mybir.dt
    f32 = dt.float32
    bf16 = dt.bfloat16
    Exp = mybir.ActivationFunctionType.Exp

    nc = bacc.Bacc("TRN2", target_bir_lowering=False, debug=False)

    xT_d = nc.dram_tensor("xT", [B, F, P], bf16, kind="ExternalInput").ap()
    str_d = nc.dram_tensor("strT", [B, HPC, S, S], bf16, kind="ExternalInput").ap()
    wq_d = nc.dram_tensor("wq", [F, F], bf16, kind="ExternalInput").ap()
    wk_d = nc.dram_tensor("wk", [F, F], bf16, kind="ExternalInput").ap()
    wv_d = nc.dram_tensor("wv", [F, F], bf16, kind="ExternalInput").ap()
    wo_d = nc.dram_tensor("wo", [P, F], bf16, kind="ExternalInput").ap()
    bias_d = nc.dram_tensor("bqkv", [3, F], bf16, kind="ExternalInput").ap()
    ident_d = nc.dram_tensor("ident", [P, P], bf16, kind="ExternalInput").ap()
    out_d = nc.dram_tensor("outT", [B, F, S], bf16, kind="ExternalOutput").ap()
    dbg = {}
    if debug_dump:
        for nm, shp, dty in [("d_qkvc0", [P, F], bf16), ("d_v20", [P, 8, P], bf16),
                             ("d_rbc0", [P, S], bf16),
                             ("d_ej0", [P, S], bf16), ("d_ot0", [P, S], bf16)]:
            dbg[nm] = nc.dram_tensor(nm, shp, dty, kind="ExternalOutput").ap()

    with ExitStack() as ctx:
        tc = ctx.enter_context(tile.TileContext(nc))
        consts = ctx.enter_context(tc.tile_pool(name="consts", bufs=1))
        qtkt = ctx.enter_context(tc.tile_pool(name="qtkt", bufs=1))
        v2p = ctx.enter_context(tc.tile_pool(name="v2", bufs=1))
        outp = ctx.enter_context(tc.tile_pool(name="outp", bufs=1))
        wop = ctx.enter_context(tc.tile_pool(name="wop", bufs=1))
        dramp = ctx.enter_context(tc.tile_pool(name="dram", bufs=1, space="DRAM"))
        xtp = ctx.enter_context(tc.tile_pool(name="xt", bufs=1))
        wp = ctx.enter_context(tc.tile_pool(name="wpool", bufs=1))
        qkvcp = ctx.enter_context(tc.tile_pool(name="qkvc", bufs=1))
        emp = ctx.enter_context(tc.tile_pool(name="em", bufs=1))
        epool = ctx.enter_context(tc.tile_pool(name="ep", bufs=3))
        miscp = ctx.enter_context(tc.tile_pool(name="misc", bufs=2))
        osp = ctx.enter_context(tc.tile_pool(name="os", bufs=3))
        psp = ctx.enter_context(tc.tile_pool(name="ps", bufs=1, space="PSUM"))

        ident = consts.tile([P, P], bf16, tag="ident", name="ident")
        nc.sync.dma_start(out=ident, in_=ident_d)
        ones_all = consts.tile([P, P], bf16, tag="ones", name="ones")
        nc.vector.memset(ones_all, 1.0)
        ones_f32 = consts.tile([P, P], f32, tag="ones32", name="ones32")
        nc.vector.memset(ones_f32, 1.0)
        bias_sb = consts.tile([1, 3 * F], bf16, tag="bias", name="bias")
        nc.sync.dma_start(out=bias_sb, in_=bias_d.rearrange("a b -> (a b)").unsqueeze(0))
        wo_sb = wop.tile([P, F], bf16, tag="wo", name="wo")
        nc.sync.dma_start(out=wo_sb, in_=wo_d)

        QT, KT, V2, OT = {}, {}, {}, {}
        for b in range(B):
            # raw transpose-output order: free index = 64*cb + r  (q = r*16+cb)
            QT[b] = qtkt.tile([P, S], bf16, tag=f"qt{b}", name=f"qt{b}")
            KT[b] = qtkt.tile([P, S], bf16, tag=f"kt{b}", name=f"kt{b}")
            OT[b] = outp.tile([P, S], bf16, tag=f"ot{b}", name=f"ot{b}")
            for hp in range(HPC):
                V2[b, hp] = v2p.tile([P, 8, P], bf16, tag=f"v{b}{hp}", name=f"v{b}{hp}")

        def q_ap(tile_, hp, qlo, qn):
            """[64, qn] view of raw-layout QT/KT covering q in [qlo, qlo+qn),
            free dims ordered (r, cb): addr = 64*cb + r, q = r*16 + cb."""
            assert qlo % 16 == 0 and qn % 16 == 0
            r0, rn = qlo // 16, qn // 16
            return tile_[64 * hp:64 * hp + 64, :].rearrange(
                "p (cb r) -> p r cb", cb=16)[:, r0:r0 + rn, :]

        # ---------- phase 1: projections + layout shuffles ----------
        xt = {}
        for b in range(B):
            xt[b] = xtp.tile([P, 8, P], bf16, tag=f"xt{b}", name=f"xt{b}")
            nc.sync.dma_start(out=xt[b], in_=xT_d[b].rearrange("(kc p) r -> p kc r", p=P))

        qkvc = {}
        for t_i, w_d in enumerate([wq_d, wk_d, wv_d]):
            wt = []
            for i in range(8):
                w_tile = wp.tile([P, F], bf16, tag=f"w{i}", name=f"w{i}")
                nc.sync.dma_start(out=w_tile, in_=w_d[i * P:(i + 1) * P, :])
                wt.append(w_tile)
            for b in range(B):
                cc = qkvcp.tile([P, F], bf16, tag=f"c{b}", name=f"c{t_i}{b}")
                qkvc[t_i, b] = cc
                for h2 in range(2):
                    ps = psp.tile([P, 512], f32, tag="mm", bufs=3, name="pj")
                    for kc in range(8):
                        nc.tensor.matmul(
                            ps, xt[b][:, kc, :],
                            wt[kc][:, 512 * h2:512 * (h2 + 1)],
                            start=(kc == 0), stop=False)
                    nc.tensor.matmul(
                        ps, ones_all[0:1, :],
                        bias_sb[0:1, 1024 * t_i + 512 * h2:1024 * t_i + 512 * h2 + 512],
                        start=False, stop=True)
                    nc.scalar.copy(cc[:, 512 * h2:512 * (h2 + 1)], ps)
                if t_i < 2:
                    # transposes for Q (t_i=0) / K (t_i=1) right away.
                    # QT keeps raw [cb, r] order (contiguous copy; the QK
                    # matmul reads it as the moving operand through a 2-dim
                    # AP). KT must be q-ordered: it is the stationary operand
                    # and walrus requires a single free dim there.
                    dstmap = QT if t_i == 0 else KT
                    for half in range(2):
                        for hp in range(HPC):
                            base = 64 * hp
                            pst = psp.tile([P, 512], bf16, tag="mm", bufs=3,
                                           name="tp")
                            for cb8 in range(8):
                                cb = 8 * half + cb8
                                nc.tensor.transpose(
                                    pst[0:64, 64 * cb8:64 * cb8 + 64],
                                    cc[base:base + 64, 64 * cb:64 * cb + 64],
                                    ident[base:base + 64, base:base + 64])
                            if t_i == 0:
                                nc.vector.tensor_copy(
                                    dstmap[b][64 * hp:64 * hp + 64,
                                              512 * half:512 * (half + 1)],
                                    pst[0:64, :])
                            else:
                                dst = dstmap[b][64 * hp:64 * hp + 64, :].rearrange(
                                    "p (r cb) -> p cb r", cb=16)[:, 8 * half:8 * half + 8, :]
                                nc.vector.tensor_copy(
                                    dst, pst[0:64, :].rearrange(
                                        "p (cb8 r) -> p cb8 r", cb8=8))
                else:
                    vs = dramp.tile([P, F], bf16, tag=f"vs{b}", name=f"vs{b}")
                    nc.sync.dma_start(out=vs, in_=cc[:])
                    # V2 column layout: col 0 = ones (PV row-sum at partition 0),
                    # cols 64:128 = V values.
                    for hp in range(HPC):
                        nc.vector.memset(V2[b, hp], 0.0)
                        src = vs[64 * hp:64 * hp + 64, :].rearrange(
                            "(j r) (cb d) -> (r cb) j d", j=8, cb=16)
                        nc.sync.dma_start(out=V2[b, hp][:, :, 64:128], in_=src)
                        nc.vector.memset(V2[b, hp][:, :, 0:1], 1.0)

        if debug_dump:
            nc.sync.dma_start(out=dbg["d_qkvc0"], in_=qkvc[0, 0][:])
            nc.sync.dma_start(out=dbg["d_v20"], in_=V2[0, 0][:])

        # ---------- phase 2+3 per batch ----------
        for b in range(B):
            eM = {}
            for hp in range(HPC):
                ps_r1 = [psp.tile([1, 512], f32, tag=f"r1_{h2}", name=f"r1_{h2}")
                         for h2 in range(2)]
                for j in range(8):
                    jl = 128 * j if causal else 0
                    w = S - jl
                    t = emp.tile([P, w], bf16, tag=f"e{hp}{j}", name=f"e{hp}{j}")
                    eM[hp, j] = t
                    nc.sync.dma_start(out=t, in_=str_d[b, hp, 128 * j:128 * (j + 1), jl:])
                    nc.scalar.activation(t, t, Exp)
                    for h2 in range(2):
                        lo = max(512 * h2, jl)
                        hi = 512 * (h2 + 1)
                        if lo < hi:
                            last_j = (3 if h2 == 0 else 7) if causal else 7
                            nc.tensor.matmul(
                                ps_r1[h2][0:1, lo - 512 * h2:hi - 512 * h2],
                                ones_all[:, 0:1], t[:, lo - jl:hi - jl],
                                start=(j == 0), stop=(j == last_j))
                r1sb = miscp.tile([1, S], f32, tag=f"r1sb{hp}", name=f"r1sb{hp}")
                rbc = miscp.tile([P, S], bf16, tag=f"rbc{hp}", name=f"rbc{hp}")
                for h2 in range(2):
                    sl = slice(512 * h2, 512 * (h2 + 1))
                    nc.vector.reciprocal_approx_fast(r1sb[:, sl], ps_r1[h2])
                    psb = psp.tile([P, 512], f32, tag="r1b", name="r1b")
                    nc.tensor.matmul(psb, ones_f32[0:1, :], r1sb[0:1, sl],
                                     start=True, stop=True)
                    nc.vector.tensor_copy(rbc[:, sl], psb)
                if debug_dump and b == 0 and hp == 0:
                    nc.sync.dma_start(out=dbg["d_rbc0"], in_=rbc[:])
                eM[hp, "rbc"] = rbc

            for hp in range(HPC):
                rbc = eM[hp, "rbc"]
                pv = [psp.tile([P, 512], f32, tag=f"pv{h2}", name=f"pv{h2}")
                      for h2 in range(2)]
                for j in range(8):
                    jl = 128 * j if causal else 0
                    Ej = epool.tile([P, S], bf16, tag="E", name="E")
                    for h2 in range(2):
                        lo_h, hi_h = 512 * h2, 512 * (h2 + 1)
                        Esc = epool.tile([P, 512], f32, tag="Esc", name="Esc")
                        qk = psp.tile([P, 512], f32, tag="mm", bufs=3, name="qk")
                        nc.tensor.matmul(
                            qk, KT[b][64 * hp:64 * hp + 64, 128 * j:128 * (j + 1)],
                            q_ap(QT[b], hp, lo_h, 512),
                            start=True, stop=True)
                        m0_hi = min(jl, hi_h)
                        if m0_hi > lo_h:
                            nc.scalar.activation(Ej[:, lo_h:m0_hi],
                                                 qk[:, 0:m0_hi - lo_h], Exp)
                        v_lo = max(jl, lo_h)
                        if v_lo < hi_h:
                            w2 = hi_h - v_lo
                            # sm = eM * (1/r1)  (bf16 in, f32 out to match qk)
                            nc.vector.tensor_mul(
                                Esc[:, 0:w2], eM[hp, j][:, v_lo - jl:hi_h - jl],
                                rbc[:, v_lo:hi_h])
                            nc.vector.tensor_add(
                                Esc[:, 0:w2], Esc[:, 0:w2],
                                qk[:, v_lo - lo_h:hi_h - lo_h])
                            nc.scalar.activation(Ej[:, v_lo:hi_h], Esc[:, 0:w2], Exp)
                        nc.tensor.matmul(pv[h2], V2[b, hp][:, j, :], Ej[:, lo_h:hi_h],
                                         start=(j == 0), stop=(j == 7))
                    if debug_dump and b == 0 and hp == 0 and j == 0:
                        nc.sync.dma_start(out=dbg["d_ej0"], in_=Ej[:])
                # normalize rows of PV by 1/rowsum2 (ones column -> PSUM row 0)
                dlo = 64 * hp
                r2sb = miscp.tile([1, S], f32, tag="r2sb", name="r2sb")
                for h2 in range(2):
                    sl = slice(512 * h2, 512 * (h2 + 1))
                    nc.vector.reciprocal_approx_fast(r2sb[0:1, sl], pv[h2][0:1, :])
                    psb = psp.tile([P, 512], f32, tag="r1b", name="bc")
                    nc.tensor.matmul(psb[dlo:dlo + 64, :],
                                     ones_f32[0:1, 0:64], r2sb[0:1, sl],
                                     start=True, stop=True)
                    r2bc = miscp.tile([P, 512], f32, tag="r2bc", name="r2bc")
                    nc.vector.tensor_copy(r2bc[dlo:dlo + 64, :], psb[dlo:dlo + 64, :])
                    nc.vector.tensor_mul(OT[b][dlo:dlo + 64, sl], pv[h2][64:128, :],
                                         r2bc[dlo:dlo + 64, :])

            if debug_dump and b == 0:
                nc.sync.dma_start(out=dbg["d_ot0"], in_=OT[0][:])

            # output projection for this batch (overlaps next batch's attention)
            for fo in range(8):
                ot = osp.tile([P, S], bf16, tag="os", name="os")
                for h2 in range(2):
                    ps = psp.tile([P, 512], f32, tag="mm", bufs=3, name="op")
                    nc.tensor.matmul(ps, wo_sb[:, 128 * fo:128 * (fo + 1)],
                                     OT[b][:, 512 * h2:512 * (h2 + 1)],
                                     start=True, stop=True)
                    if h2 == 0:
                        nc.scalar.copy(ot[:, 512 * h2:512 * (h2 + 1)], ps)
                    else:
                        nc.vector.tensor_copy(ot[:, 512 * h2:512 * (h2 + 1)], ps)
                nc.sync.dma_start(out=out_d[b, 128 * fo:128 * (fo + 1), :], in_=ot)

    nc.compile()
    return nc


def _prep_host(x, str_mat, attn_mask, Wq, bq, Wk, bk, Wv, bv, Wo, bo):
    import ml_dtypes
    bf = ml_dtypes.bfloat16

    x = np.asarray(x, np.float32)
    str_mat = np.asarray(str_mat, np.float32)
    attn_mask = np.asarray(attn_mask, np.float32)
    mask = attn_mask[:, 0]  # [b, s, s]
    causal = bool((mask == np.tril(np.ones((S, S), np.float32))[None]).all())
    strT = np.where(mask[:, None] == 0.0, NEG_FILL, str_mat).transpose(0, 1, 3, 2)
    strT = strT.astype(bf)
    xT = x.transpose(0, 2, 1).astype(bf)  # [b, f, s]
    Wq_s = (np.asarray(Wq, np.float32) / D).astype(bf)
    bq_s = (np.asarray(bq, np.float32) / D)
    bias = np.stack([bq_s, np.asarray(bk, np.float32),
                     np.asarray(bv, np.float32)]).astype(bf)
    Wk_b = np.asarray(Wk, np.float32).astype(bf)
    Wv_b = np.asarray(Wv, np.float32).astype(bf)
    Wo_b = np.asarray(Wo, np.float32).astype(bf)
    ident = np.eye(P, dtype=bf)
    in_maps = []
    for c in range(NCORES):
        in_maps.append({
            "xT": np.ascontiguousarray(xT[:, :, P * c:P * (c + 1)]),
            "strT": np.ascontiguousarray(strT[:, HPC * c:HPC * (c + 1)]),
            "wq": Wq_s, "wk": Wk_b, "wv": Wv_b,
            "wo": np.ascontiguousarray(Wo_b[P * c:P * (c + 1)]),
            "bqkv": bias, "ident": ident,
        })
    return in_maps, causal


def kernel(**inputs):
    from concourse.bass_utils import run_bass_kernel_spmd

    in_maps, causal = _prep_host(**inputs)
    if causal not in _CACHE:
        _CACHE[causal] = _build_nc(causal=causal)
    nc = _CACHE[causal]
    res = run_bass_kernel_spmd(nc, in_maps, core_ids=list(range(NCORES)))
    partials = [r["outT"].astype(np.float32) for r in res.results]
    out = np.sum(partials, axis=0, dtype=np.float32)  # [b, f, s]
    out = out.transpose(0, 2, 1) + np.asarray(inputs["bo"], np.float32)
    return np.ascontiguousarray(out.astype(np.float32))
